# revision 1
# baseline (speedup 1.0000x reference)
"""AttentionNet kernel for Trainium2: 8-core data-parallel over batch.

Reference computation (per batch element b):
  emb    = x.reshape(N,64) @ conv_w + conv_b          [N,512]
  x_real = emb * mask[:,None]
  query  = sum_n(x_real) / (sum(mask)+1e-5)           [512]
  q_proj = query @ Uq                                 [512]
  r_proj = x_real @ Ur                                [N,512]
  logits = tanh(q_proj + r_proj) @ Ua                 [N]
  attn   = softmax(logits masked)                     [N]
  out    = attn @ x_real                              [512]

Kernel restructure (per core, batch shard of 256, fp16 data path):
  * host prep (layout only): xmT = (x*mask) transposed to [64, B*N] fp16.
  * conv fold: r_proj = xm @ (W@Ur) + mask*(b@Ur);  q_proj enters the same
    matmul through 4 per-batch indicator rows (rank-4 update), so
    z = r_proj + q_proj is ONE K=69 matmul per (k-chunk, macro).
  * xaT[128, B*N] rows 0:64 = xmT (direct contiguous DMAs), row 64 = mask,
    rows 65:69 = indicators (one broadcast DMA).  No on-device transposes.
  * xa_sum = grouped DVE reduce of xaT (row 64 gives sum(mask) for free).
  * logits = Ua_rep.T @ tanh(z) with Ua replicated across 128 cols ->
    logits replicated across partitions -> exp() broadcast is free.
  * unnormalized softmax: e = exp(logits-2); weighted reduce of xaT rows
    0:65 by e gives esum and Z = sum(e*mask) (row 64) in one pass; masked
    columns of xaT are zero so they drop out automatically.
  * out = (esum.T @ [W;b]) * (1/Z)  (fp32 finish)
"""

import os
import sys

sys.path.insert(0, "/opt/trn_rl_repo")

import numpy as np
from contextlib import ExitStack

import concourse.bass as bass
import concourse.bacc as bacc
import concourse.tile as tile
from concourse import mybir

B, N, DOBJ, DM = 2048, 128, 64, 512
NCORES = 8
BSH = B // NCORES          # 256 batch per core
MB = 4                     # batch elements per macro tile
NMAC = BSH // MB           # 64 macro tiles
HM = NMAC // 2             # 32 macros per half
R = MB * N                 # 512 rows per macro
KC = 4                     # 512 = 4 chunks of 128 along d_model
XCHUNK = 4                 # macros per xmT load DMA
F32 = mybir.dt.float32
F16 = mybir.dt.float16
AF = mybir.ActivationFunctionType
ALU = mybir.AluOpType
AX = mybir.AxisListType
EXP_SHIFT = -2.0           # exp(logits+shift): keeps e in fp16 range


def build_nc():
    nc = bacc.Bacc("TRN2", target_bir_lowering=False, debug=False, num_devices=1)

    xmt = nc.dram_tensor("xmt", [DOBJ, BSH * N], F16, kind="ExternalInput")
    mask = nc.dram_tensor("mask", [BSH, N], F16, kind="ExternalInput")
    ind = nc.dram_tensor("ind4", [MB, R], F16, kind="ExternalInput")
    w = nc.dram_tensor("conv_w", [DOBJ, DM], F32, kind="ExternalInput")
    cb = nc.dram_tensor("conv_b", [1, DM], F32, kind="ExternalInput")
    uq = nc.dram_tensor("Uq", [DM, DM], F32, kind="ExternalInput")
    ur = nc.dram_tensor("Ur", [DM, DM], F32, kind="ExternalInput")
    ua = nc.dram_tensor("ua", [1, DM], F32, kind="ExternalInput")
    ident = nc.dram_tensor("ident", [128, 128], F32, kind="ExternalInput")
    out = nc.dram_tensor("out", [BSH, DM], F32, kind="ExternalOutput")

    # persistent SBUF
    xaT = nc.alloc_sbuf_tensor("xaT", [69, NMAC * R], F16).ap()    # 64KB/part
    wb = nc.alloc_sbuf_tensor("wb", [65, DM], F32).ap()            # [[W];[b]]
    wura = nc.alloc_sbuf_tensor("wura", [65, DM], F32).ap()
    wauq = nc.alloc_sbuf_tensor("wauq", [65, DM], F16).ap()
    uarep = nc.alloc_sbuf_tensor("uarep", [128, DM], F16).ap()
    recipd = nc.alloc_sbuf_tensor("recipd", [128, 8], F32).ap()
    recipz = nc.alloc_sbuf_tensor("recipz", [128, 2], F32).ap()
    qptt = [nc.alloc_sbuf_tensor(f"qptt{i}", [32, 2 * DM], F16).ap()
            for i in range(4)]                                     # [b, k]
    xasum = nc.alloc_sbuf_tensor("xasum", [65, BSH], F16).ap()
    xaesum = nc.alloc_sbuf_tensor("xaesum", [65, BSH], F32).ap()
    rpw = nc.alloc_sbuf_tensor("rpw", [69, 2 * DM], F16).ap()      # lhsT ring
    id_sb = nc.alloc_sbuf_tensor("id_sb", [128, 128], F32).ap()
    ua_nat = nc.alloc_sbuf_tensor("ua_nat", [1, DM], F32).ap()
    wbt = nc.alloc_sbuf_tensor("wbt", [128, 4 * 65], F32).ap()
    eshift = nc.alloc_sbuf_tensor("eshift", [128, 1], F32).ap()

    with tile.TileContext(nc) as tc:
        # ---------------- setup: loads ----------------
        nc.sync.dma_start(out=id_sb, in_=ident.ap())
        nc.vector.memset(eshift, EXP_SHIFT)
        nc.sync.dma_start(out=wb[0:64, :], in_=w.ap())
        nc.sync.dma_start(out=wb[64:65, :], in_=cb.ap())
        nc.sync.dma_start(out=ua_nat, in_=ua.ap())
        # mask row + indicator rows of xaT
        nc.sync.dma_start(
            out=xaT[64:65, :],
            in_=bass.AP(tensor=mask, offset=0, ap=[[0, 1], [1, BSH * N]]),
        )
        nc.sync.dma_start(
            out=xaT[65:69, :].rearrange("p (m r) -> p m r", r=R),
            in_=bass.AP(tensor=ind, offset=0, ap=[[R, MB], [0, NMAC], [1, R]]),
        )
        # xmT rows: big contiguous DMAs, XCHUNK macros each
        for c in range(NMAC // XCHUNK):
            eng = nc.sync if c % 2 == 0 else nc.gpsimd
            eng.dma_start(
                out=xaT[0:64, c * XCHUNK * R:(c + 1) * XCHUNK * R],
                in_=bass.AP(
                    tensor=xmt, offset=c * XCHUNK * R,
                    ap=[[BSH * N, DOBJ], [1, XCHUNK * R]],
                ),
            )

        with ExitStack() as ctx:
            sps = ctx.enter_context(tc.tile_pool(name="sps", bufs=2, space="PSUM"))
            ssb = ctx.enter_context(tc.tile_pool(name="ssb", bufs=4))

            # W.T chunks for the weight-fold matmuls
            for mc in range(KC):
                tp = sps.tile([128, 65], F32, tag="tp")
                nc.tensor.transpose(
                    tp, wb[:, mc * 128:(mc + 1) * 128], id_sb[0:65, 0:65]
                )
                nc.vector.tensor_copy(out=wbt[:, mc * 65:(mc + 1) * 65], in_=tp)

            urt, uqt = [], []
            for mc in range(KC):
                t1 = ssb.tile([128, DM], F32, tag="urt")
                nc.sync.dma_start(out=t1, in_=ur.ap()[mc * 128:(mc + 1) * 128, :])
                urt.append(t1)
                t2 = ssb.tile([128, DM], F32, tag="uqt")
                nc.sync.dma_start(out=t2, in_=uq.ap()[mc * 128:(mc + 1) * 128, :])
                uqt.append(t2)

            wura_ps = sps.tile([65, DM], F32, tag="wu")
            for mc in range(KC):
                nc.tensor.matmul(
                    wura_ps, wbt[:, mc * 65:(mc + 1) * 65], urt[mc],
                    start=(mc == 0), stop=(mc == KC - 1),
                )
            nc.vector.tensor_copy(out=wura, in_=wura_ps)
            wauq_ps = sps.tile([65, DM], F32, tag="wu")
            for mc in range(KC):
                nc.tensor.matmul(
                    wauq_ps, wbt[:, mc * 65:(mc + 1) * 65], uqt[mc],
                    start=(mc == 0), stop=(mc == KC - 1),
                )
            nc.vector.tensor_copy(out=wauq, in_=wauq_ps)

            # static rows of the r_proj lhsT ring (both parities), fp16
            for p in range(2):
                nc.scalar.copy(out=rpw[0:65, p * DM:(p + 1) * DM], in_=wura)

            # Ua replicated chunks (fp16)
            for kc in range(KC):
                uac_ps = sps.tile([128, 1], F32, tag="tp")
                nc.tensor.transpose(
                    uac_ps, ua_nat[0:1, kc * 128:(kc + 1) * 128], id_sb[0:1, 0:1]
                )
                uac = ssb.tile([128, 1], F32, tag="uac")
                nc.vector.tensor_copy(out=uac, in_=uac_ps)
                nc.vector.tensor_copy(
                    out=uarep[:, kc * 128:(kc + 1) * 128],
                    in_=uac.broadcast_to((128, 128)),
                )

        # ---------------- main: two halves, pipelined ----------------
        with ExitStack() as ctx:
            qps = ctx.enter_context(tc.tile_pool(name="qps", bufs=2, space="PSUM"))
            rps = ctx.enter_context(tc.tile_pool(name="rps", bufs=2, space="PSUM"))
            lps = ctx.enter_context(tc.tile_pool(name="lps", bufs=1, space="PSUM"))
            qsb = ctx.enter_context(tc.tile_pool(name="qsb", bufs=2))
            zsb = ctx.enter_context(tc.tile_pool(name="zsb", bufs=3))
            esb = ctx.enter_context(tc.tile_pool(name="esb", bufs=2))
            fsb = ctx.enter_context(tc.tile_pool(name="fsb", bufs=2))

            NG = 8                      # groups
            GM = NMAC // NG             # 8 macros per group
            for g in range(NG):
                h = (g * GM) // HM
                gb0 = g * GM * MB       # first batch of group
                boff = gb0 % 128
                # ---- phase 1: xa_sum for this group ----
                for mi in range(GM):
                    m = g * GM + mi
                    b0 = m * MB
                    with nc.allow_low_precision(reason="fp16 xa_sum"):
                        nc.vector.reduce_sum(
                            out=xasum[0:65, b0:b0 + MB],
                            in_=xaT[0:65, m * R:(m + 1) * R].rearrange(
                                "p (g n) -> p g n", n=N
                            ),
                            axis=AX.X,
                        )

                # denom reciprocal for this group (from xasum row 64)
                GBS = GM * MB           # 32 batch per group
                zc = fsb.tile([128, 1], F32, tag="dzc")
                nc.gpsimd.dma_start(
                    out=zc[0:GBS, :], in_=xasum[64:65, gb0:gb0 + GBS]
                )
                zc2 = fsb.tile([128, 1], F32, tag="dzc2")
                nc.vector.tensor_scalar(
                    out=zc2[0:GBS, :], in0=zc[0:GBS, :], scalar1=1e-5,
                    scalar2=None, op0=ALU.add,
                )
                nc.vector.reciprocal(
                    out=recipd[0:GBS, g:g + 1], in_=zc2[0:GBS, :]
                )

                # ---- q_proj for this group: out = xasum_g.T @ wauq_kc
                # gives [b, k] directly (no transpose needed)
                for kc in range(KC):
                    qp_ps = qps.tile([GBS, 128], F32, tag="qp")
                    nc.tensor.matmul(
                        qp_ps, xasum[:, gb0:gb0 + GBS],
                        wauq[:, kc * 128:(kc + 1) * 128],
                        start=True, stop=True,
                    )
                    nc.vector.tensor_copy(
                        out=qptt[(g % 4)][:,
                                 h * DM + kc * 128: h * DM + (kc + 1) * 128],
                        in_=qp_ps,
                    )
                nc.vector.tensor_scalar(
                    out=qptt[g % 4][:, h * DM:(h + 1) * DM],
                    in0=qptt[g % 4][:, h * DM:(h + 1) * DM],
                    scalar1=recipd[0:GBS, g:g + 1], scalar2=None, op0=ALU.mult,
                )

                # ---- phase 2: attention for this group (macro pairs) ----
                for ti in range(GM // 2):
                    logits_ps = lps.tile([128, 2 * R], F32, tag="lg")
                    subs = [g * GM + 2 * ti, g * GM + 2 * ti + 1]
                    for si, s in enumerate(subs):
                        b0 = s * MB
                        sboff = b0 % 128
                        par = s % 2
                        blk = sboff // 32
                        brow = sboff % 32
                        nc.gpsimd.dma_start(
                            out=rpw[65:69, par * DM:(par + 1) * DM],
                            in_=qptt[blk][brow:brow + MB,
                                          h * DM:(h + 1) * DM],
                        )
                        for half2 in range(2):
                            rp_ps = rps.tile([128, 2 * R], F32, tag="rp")
                            for k2 in range(2):
                                kc = half2 * 2 + k2
                                nc.tensor.matmul(
                                    rp_ps[:, k2 * R:(k2 + 1) * R],
                                    rpw[:, par * DM + kc * 128:
                                        par * DM + (kc + 1) * 128],
                                    xaT[0:69, s * R:(s + 1) * R],
                                    start=True, stop=True,
                                )
                            zt = zsb.tile([128, 2 * R], F16, tag="zt")
                            nc.scalar.activation(out=zt, in_=rp_ps, func=AF.Tanh)
                            for k2 in range(2):
                                kc = half2 * 2 + k2
                                nc.tensor.matmul(
                                    logits_ps[:, si * R:(si + 1) * R],
                                    uarep[:, kc * 128:(kc + 1) * 128],
                                    zt[:, k2 * R:(k2 + 1) * R],
                                    start=(kc == 0), stop=(kc == KC - 1),
                                )
                    e_sb = esb.tile([65, 2 * R], F16, tag="e")
                    nc.scalar.activation(
                        out=e_sb, in_=logits_ps[0:65, :], func=AF.Exp,
                        bias=eshift[0:65, :],
                    )
                    prod = esb.tile([65, 2 * R], F16, tag="prod")
                    nc.vector.tensor_tensor(
                        out=prod,
                        in0=xaT[0:65, subs[0] * R:(subs[0] + 2) * R],
                        in1=e_sb, op=ALU.mult,
                    )
                    with nc.allow_low_precision(reason="fp16 prod"):
                        nc.vector.reduce_sum(
                            out=xaesum[0:65, subs[0] * MB:(subs[0] + 2) * MB],
                            in_=prod.rearrange("p (g n) -> p g n", n=N),
                            axis=AX.X,
                        )

        # ---------------- final: normalize + output ----------------
        with ExitStack() as ctx:
            fps = ctx.enter_context(tc.tile_pool(name="fps", bufs=2, space="PSUM"))
            f2sb = ctx.enter_context(tc.tile_pool(name="f2sb", bufs=2))
            for h in range(2):
                zc = f2sb.tile([128, 1], F32, tag="zc")
                nc.sync.dma_start(
                    out=zc, in_=xaesum[64:65, h * 128:(h + 1) * 128]
                )
                zc2 = f2sb.tile([128, 1], F32, tag="zc2")
                nc.vector.tensor_scalar(
                    out=zc2, in0=zc, scalar1=1e-30, scalar2=None, op0=ALU.add
                )
                nc.vector.reciprocal(out=recipz[:, h:h + 1], in_=zc2)
            for h in range(2):
                out_ps = fps.tile([128, DM], F32, tag="op")
                nc.tensor.matmul(
                    out_ps, xaesum[0:65, h * 128:(h + 1) * 128], wb,
                    start=True, stop=True,
                )
                out_sb = f2sb.tile([128, DM], F32, tag="ob")
                nc.vector.tensor_scalar(
                    out=out_sb, in0=out_ps, scalar1=recipz[:, h:h + 1],
                    scalar2=None, op0=ALU.mult,
                )
                nc.sync.dma_start(
                    out=out.ap()[h * 128:(h + 1) * 128, :], in_=out_sb
                )

    nc.compile()
    return nc


def prep_core_inputs(x_shard, mask_shard):
    """Host-side layout prep for one core: fp16 cast + mask + transpose."""
    xm = (x_shard.astype(np.float32).reshape(BSH, N, DOBJ)
          * mask_shard.astype(np.float32)[:, :, None]).astype(np.float16)
    xmt = np.ascontiguousarray(xm.reshape(BSH * N, DOBJ).T)   # [64, BSH*N]
    ind4 = np.zeros((MB, R), dtype=np.float16)
    for j in range(MB):
        ind4[j, j * N:(j + 1) * N] = 1.0
    return xmt, mask_shard.astype(np.float16), ind4


def _ensure_ntff_hook():
    """Provide antenv.axon_hooks if the image lacks it (NTFF profiling via
    ctypes into libaxon_pjrt.so), and stub out the artifact upload."""
    import types
    import ctypes
    import contextlib

    try:
        from antenv.axon_hooks import get_axon_ntff_profile_hook  # noqa: F401
    except ImportError:
        so_path = "/opt/axon/libaxon_pjrt.so"
        hook = None
        if os.path.exists(so_path):
            lib = ctypes.CDLL(so_path)
            if hasattr(lib, "axon_start_nrt_profile"):
                lib.axon_start_nrt_profile.argtypes = [
                    ctypes.POINTER(ctypes.c_int64), ctypes.c_size_t,
                ]
                lib.axon_start_nrt_profile.restype = ctypes.c_int64
                lib.axon_stop_nrt_profile.argtypes = [ctypes.c_char_p]
                lib.axon_stop_nrt_profile.restype = ctypes.c_int64

                @contextlib.contextmanager
                def _hook(output_dir, device_ids):
                    import jax
                    jax.devices()
                    if device_ids:
                        ids = (ctypes.c_int64 * len(device_ids))(*device_ids)
                        rc = lib.axon_start_nrt_profile(ids, len(device_ids))
                    else:
                        rc = lib.axon_start_nrt_profile(None, 0)
                    if rc != 0:
                        raise RuntimeError(f"axon_start_nrt_profile rc={rc}")
                    try:
                        yield
                    finally:
                        n = lib.axon_stop_nrt_profile(str(output_dir).encode())
                        print(f"ntff profile: {n} file(s) -> {output_dir}",
                              file=sys.stderr)

                hook = _hook

        import antenv
        mod = types.ModuleType("antenv.axon_hooks")
        mod.get_axon_ntff_profile_hook = lambda: hook
        mod.set_axon_ntff_profile_hook = lambda h: None
        sys.modules["antenv.axon_hooks"] = mod
        antenv.axon_hooks = mod

    import concourse.bass_utils as bu
    bu.upload_artifacts = lambda tmpdir: f"file://{tmpdir}"


def kernel(x_others, x_mask, conv_w, conv_b, Uq, Ur, Ua):
    x_others = np.asarray(x_others)
    x_mask = np.asarray(x_mask)
    conv_w = np.ascontiguousarray(np.asarray(conv_w, dtype=np.float32))
    conv_b = np.asarray(conv_b, dtype=np.float32).reshape(1, DM)
    Uq = np.ascontiguousarray(np.asarray(Uq, dtype=np.float32))
    Ur = np.ascontiguousarray(np.asarray(Ur, dtype=np.float32))
    Ua = np.asarray(Ua, dtype=np.float32).reshape(1, DM)
    ident = np.eye(128, dtype=np.float32)

    nc = build_nc()

    in_maps = []
    for c in range(NCORES):
        sl = slice(c * BSH, (c + 1) * BSH)
        xmt, m16, ind4 = prep_core_inputs(x_others[sl], x_mask[sl])
        in_maps.append({
            "xmt": xmt,
            "mask": np.ascontiguousarray(m16),
            "ind4": ind4,
            "conv_w": conv_w,
            "conv_b": conv_b,
            "Uq": Uq,
            "Ur": Ur,
            "ua": Ua,
            "ident": ident,
        })

    from concourse.bass_utils import run_bass_kernel_spmd

    trace = os.environ.get("KERNEL_TRACE", "0") == "1"
    if trace:
        _ensure_ntff_hook()
    tmpdir = None
    if trace:
        import tempfile
        os.makedirs("/root/problem/traces", exist_ok=True)
        tmpdir = tempfile.mkdtemp(dir="/root/problem/traces")
        print(f"trace dir: {tmpdir}", file=sys.stderr)
    res = run_bass_kernel_spmd(
        nc, in_maps, core_ids=list(range(NCORES)), trace=trace, tmpdir=tmpdir
    )
    if trace and res.exec_time_ns is not None:
        print(f"HW exec time: {res.exec_time_ns} ns", file=sys.stderr)
        kernel.last_exec_time_ns = res.exec_time_ns
        kernel.last_trace = res.instructions_and_trace
    out = np.concatenate([r["out"] for r in res.results], axis=0)
    return out


if __name__ == "__main__":
    rng = np.random.default_rng(0)
    x = rng.standard_normal((B, N * DOBJ), dtype=np.float32)
    mask = rng.integers(0, 2, (B, N)).astype(np.float32)
    w = rng.standard_normal((DOBJ, DM), dtype=np.float32) / 8.0
    cbv = np.zeros((DM,), dtype=np.float32)
    uq = rng.standard_normal((DM, DM), dtype=np.float32) / 22.6
    urm = rng.standard_normal((DM, DM), dtype=np.float32) / 22.6
    uav = rng.standard_normal((DM,), dtype=np.float32) * 0.1
    out = kernel(x, mask, w, cbv, uq, urm, uav)
    print(out.shape, out.dtype)



# revision 4
# speedup vs baseline: 1.6096x; 1.6096x over previous
"""AttentionNet kernel for Trainium2: 8-core data-parallel over batch.

Reference computation (per batch element b):
  emb    = x.reshape(N,64) @ conv_w + conv_b          [N,512]
  x_real = emb * mask[:,None]
  query  = sum_n(x_real) / (sum(mask)+1e-5)           [512]
  q_proj = query @ Uq                                 [512]
  r_proj = x_real @ Ur                                [N,512]
  logits = tanh(q_proj + r_proj) @ Ua                 [N]
  attn   = softmax(logits masked)                     [N]
  out    = attn @ x_real                              [512]

Kernel restructure (v2: masked-column packing, fp16 data path):
  * Masked positions contribute nothing (x_real = 0 there), and the output
    is permutation-invariant in n.  Host packs each batch's VALID columns
    first (the rest are zero), sorts all B batches by valid count, and
    groups them into tiles of TB in {8,4} batches padded to the tile max
    width w.  Tile plan is GLOBAL (shared by all 8 cores; core k takes the
    k-th contiguous TB-slice of each sorted 64/32-batch chunk), so one
    compiled kernel serves all cores.  Columns drop ~2x vs dense N=128.
  * conv fold: r_proj = xm @ (W@Ur) + mask*(b@Ur); q_proj enters the same
    matmul through per-tile indicator rows (rank-TB update), so
    z = r_proj + q_proj is ONE K=73 matmul per (k-chunk, tile).
  * xaT[73, V] rows 0:64 = xmT (contiguous DMAs), row 64 = mask,
    rows 65:73 = indicators.  No on-device transposes.
  * xasum = grouped DVE reduce of xaT per tile (row 64 = sum(mask) free).
  * logits = Ua_rep.T @ tanh(z) with Ua replicated across 128 cols ->
    logits replicated across partitions -> exp() broadcast is free.
  * unnormalized softmax: e = exp(logits-2); weighted reduce of xaT rows
    0:65 by e gives esum and Z = sum(e*mask) (row 64); masked/padded
    columns of xaT are zero so they drop out automatically.
  * out = (esum.T @ [W;b]) * (1/Z)  (fp32 finish); host un-permutes rows.
"""

import os
import sys

sys.path.insert(0, "/opt/trn_rl_repo")

import numpy as np
from contextlib import ExitStack

import concourse.bass as bass
import concourse.bacc as bacc
import concourse.tile as tile
from concourse import mybir

B, N, DOBJ, DM = 2048, 128, 64, 512
NCORES = 8
BSH = B // NCORES          # 256 batch per core
KC = 4                     # 512 = 4 chunks of 128 along d_model
NIND = 8                   # indicator rows (max TB)
KTOT = DOBJ + 1 + NIND     # 73 contraction rows
F32 = mybir.dt.float32
F16 = mybir.dt.float16
AF = mybir.ActivationFunctionType
ALU = mybir.AluOpType
AX = mybir.AxisListType
EXP_SHIFT = -2.0           # exp(logits+shift): keeps e in fp16 range
NCHUNK = 16                # xmT load DMA chunks


def make_plan(c):
    """Global tile plan from per-batch valid counts c[B] (any core order).

    Returns (order, plan, V): order = batches sorted by count desc;
    plan = list of (TB, w, b0, off) shared by all cores; V = packed width.
    Each plan entry consumes 8*TB consecutive sorted batches (TB per core).
    TB=8 when w <= 64 else 4, so R2 = TB*w <= 512 (one PSUM bank fp32).
    """
    order = np.argsort(-c, kind="stable")
    plan = []
    p, b0, off = 0, 0, 0
    while p < B:
        w = max(int(c[order[p]]), 1)
        TB = 8 if w <= 64 else 4
        if p + 8 * TB > B:
            TB = 4
        plan.append((TB, w, b0, off))
        p += 8 * TB
        b0 += TB
        off += TB * w
    return order, plan, off


def build_nc(plan, V):
    nc = bacc.Bacc("TRN2", target_bir_lowering=False, debug=False, num_devices=1)

    xmt = nc.dram_tensor("xmt", [DOBJ, V], F16, kind="ExternalInput")
    mask = nc.dram_tensor("mask", [1, V], F16, kind="ExternalInput")
    ind = nc.dram_tensor("ind8", [NIND, V], F16, kind="ExternalInput")
    w_t = nc.dram_tensor("conv_w", [DOBJ, DM], F32, kind="ExternalInput")
    cb = nc.dram_tensor("conv_b", [1, DM], F32, kind="ExternalInput")
    uq = nc.dram_tensor("Uq", [DM, DM], F32, kind="ExternalInput")
    ur = nc.dram_tensor("Ur", [DM, DM], F32, kind="ExternalInput")
    ua = nc.dram_tensor("ua", [1, DM], F32, kind="ExternalInput")
    ident = nc.dram_tensor("ident", [128, 128], F32, kind="ExternalInput")
    out = nc.dram_tensor("out", [BSH, DM], F32, kind="ExternalOutput")

    # persistent SBUF
    xaT = nc.alloc_sbuf_tensor("xaT", [KTOT, V], F16).ap()
    wb = nc.alloc_sbuf_tensor("wb", [65, DM], F32).ap()            # [[W];[b]]
    wura = nc.alloc_sbuf_tensor("wura", [65, DM], F32).ap()
    wauq = nc.alloc_sbuf_tensor("wauq", [65, DM], F16).ap()
    uarep = nc.alloc_sbuf_tensor("uarep", [128, DM], F16).ap()
    recipd = nc.alloc_sbuf_tensor("recipd", [128, 2], F32).ap()
    recipz = nc.alloc_sbuf_tensor("recipz", [128, 2], F32).ap()
    qpt = [nc.alloc_sbuf_tensor(f"qpt{i}", [128, DM], F16).ap()
           for i in range(2)]                                      # [b, k]
    xasum = nc.alloc_sbuf_tensor("xasum", [65, BSH], F16).ap()
    xaesum = nc.alloc_sbuf_tensor("xaesum", [65, BSH], F32).ap()
    rpw = nc.alloc_sbuf_tensor("rpw", [KTOT, 2 * DM], F16).ap()    # lhsT ring
    id_sb = nc.alloc_sbuf_tensor("id_sb", [128, 128], F32).ap()
    ua_nat = nc.alloc_sbuf_tensor("ua_nat", [1, DM], F32).ap()
    wbt = nc.alloc_sbuf_tensor("wbt", [128, 4 * 65], F32).ap()
    eshift = nc.alloc_sbuf_tensor("eshift", [128, 1], F32).ap()

    with tile.TileContext(nc) as tc:
        # ---------------- setup: loads ----------------
        nc.sync.dma_start(out=id_sb, in_=ident.ap())
        nc.vector.memset(eshift, EXP_SHIFT)
        # TB=4 tiles leave rpw rows 69:73 unwritten; clear once so the
        # zero-indicator columns multiply against 0, not uninitialized NaNs
        # (start at partition 64 for alignment; row 64 is rewritten below)
        nc.vector.memset(rpw[64:65 + NIND, :], 0.0)
        nc.sync.dma_start(out=wb[0:64, :], in_=w_t.ap())
        nc.sync.dma_start(out=wb[64:65, :], in_=cb.ap())
        nc.sync.dma_start(out=ua_nat, in_=ua.ap())
        nc.sync.dma_start(out=xaT[64:65, :], in_=mask.ap())
        nc.sync.dma_start(out=xaT[65:65 + NIND, :], in_=ind.ap())
        # xmT rows: big contiguous DMAs
        bounds = [V * i // NCHUNK for i in range(NCHUNK + 1)]
        for ci in range(NCHUNK):
            c0, c1 = bounds[ci], bounds[ci + 1]
            if c0 == c1:
                continue
            eng = nc.sync if ci % 2 == 0 else nc.gpsimd
            eng.dma_start(
                out=xaT[0:64, c0:c1],
                in_=bass.AP(tensor=xmt, offset=c0, ap=[[V, DOBJ], [1, c1 - c0]]),
            )

        with ExitStack() as ctx:
            sps = ctx.enter_context(tc.tile_pool(name="sps", bufs=2, space="PSUM"))
            ssb = ctx.enter_context(tc.tile_pool(name="ssb", bufs=4))

            # W.T chunks for the weight-fold matmuls
            for mc in range(KC):
                tp = sps.tile([128, 65], F32, tag="tp")
                nc.tensor.transpose(
                    tp, wb[:, mc * 128:(mc + 1) * 128], id_sb[0:65, 0:65]
                )
                nc.vector.tensor_copy(out=wbt[:, mc * 65:(mc + 1) * 65], in_=tp)

            urt, uqt = [], []
            for mc in range(KC):
                t1 = ssb.tile([128, DM], F32, tag="urt")
                nc.sync.dma_start(out=t1, in_=ur.ap()[mc * 128:(mc + 1) * 128, :])
                urt.append(t1)
                t2 = ssb.tile([128, DM], F32, tag="uqt")
                nc.sync.dma_start(out=t2, in_=uq.ap()[mc * 128:(mc + 1) * 128, :])
                uqt.append(t2)

            wura_ps = sps.tile([65, DM], F32, tag="wu")
            for mc in range(KC):
                nc.tensor.matmul(
                    wura_ps, wbt[:, mc * 65:(mc + 1) * 65], urt[mc],
                    start=(mc == 0), stop=(mc == KC - 1),
                )
            nc.vector.tensor_copy(out=wura, in_=wura_ps)
            wauq_ps = sps.tile([65, DM], F32, tag="wu")
            for mc in range(KC):
                nc.tensor.matmul(
                    wauq_ps, wbt[:, mc * 65:(mc + 1) * 65], uqt[mc],
                    start=(mc == 0), stop=(mc == KC - 1),
                )
            nc.vector.tensor_copy(out=wauq, in_=wauq_ps)

            # static rows of the r_proj lhsT ring (both parities), fp16
            for p in range(2):
                nc.scalar.copy(out=rpw[0:65, p * DM:(p + 1) * DM], in_=wura)

            # Ua replicated chunks (fp16)
            for kc in range(KC):
                uac_ps = sps.tile([128, 1], F32, tag="tp")
                nc.tensor.transpose(
                    uac_ps, ua_nat[0:1, kc * 128:(kc + 1) * 128], id_sb[0:1, 0:1]
                )
                uac = ssb.tile([128, 1], F32, tag="uac")
                nc.vector.tensor_copy(out=uac, in_=uac_ps)
                nc.vector.tensor_copy(
                    out=uarep[:, kc * 128:(kc + 1) * 128],
                    in_=uac.broadcast_to((128, 128)),
                )

        # ---------------- main ----------------
        with ExitStack() as ctx:
            zps = ctx.enter_context(tc.tile_pool(name="zps", bufs=2, space="PSUM"))
            lps = ctx.enter_context(tc.tile_pool(name="lps", bufs=2, space="PSUM"))
            zsb = ctx.enter_context(tc.tile_pool(name="zsb", bufs=3))
            esb = ctx.enter_context(tc.tile_pool(name="esb", bufs=4))
            fsb = ctx.enter_context(tc.tile_pool(name="fsb", bufs=2))

            # phase A: per-tile column sums (query numerator + denominator)
            for (TB, w, b0, off) in plan:
                with nc.allow_low_precision(reason="fp16 xasum"):
                    nc.vector.reduce_sum(
                        out=xasum[0:65, b0:b0 + TB],
                        in_=xaT[0:65, off:off + TB * w].rearrange(
                            "p (g n) -> p g n", n=w
                        ),
                        axis=AX.X,
                    )

            def emit_qproj(blk):
                zc = fsb.tile([128, 1], F32, tag="zc")
                nc.gpsimd.dma_start(
                    out=zc, in_=xasum[64:65, blk * 128:(blk + 1) * 128]
                )
                zc2 = fsb.tile([128, 1], F32, tag="zc2")
                nc.vector.tensor_scalar(
                    out=zc2, in0=zc, scalar1=1e-5, scalar2=None, op0=ALU.add
                )
                nc.vector.reciprocal(out=recipd[:, blk:blk + 1], in_=zc2)
                for kc in range(KC):
                    qp_ps = zps.tile([128, 128], F32, tag="z")
                    nc.tensor.matmul(
                        qp_ps, xasum[:, blk * 128:(blk + 1) * 128],
                        wauq[:, kc * 128:(kc + 1) * 128],
                        start=True, stop=True,
                    )
                    nc.vector.tensor_copy(
                        out=qpt[blk][:, kc * 128:(kc + 1) * 128], in_=qp_ps
                    )
                nc.vector.tensor_scalar(
                    out=qpt[blk], in0=qpt[blk],
                    scalar1=recipd[:, blk:blk + 1], scalar2=None, op0=ALU.mult,
                )

            # phase B: attention per tile
            qproj_done = [False, False]
            for ti, (TB, w, b0, off) in enumerate(plan):
                R2 = TB * w
                par = ti % 2
                if not qproj_done[0]:
                    emit_qproj(0)
                    qproj_done[0] = True
                if b0 + TB > 128 and not qproj_done[1]:
                    emit_qproj(1)
                    qproj_done[1] = True
                # q_proj rows of this tile -> rpw[65:65+TB] (split at block edge)
                dst = 65
                for blk in range(2):
                    lo = max(b0, blk * 128)
                    hi = min(b0 + TB, (blk + 1) * 128)
                    if lo < hi:
                        nc.gpsimd.dma_start(
                            out=rpw[dst:dst + (hi - lo),
                                    par * DM:(par + 1) * DM],
                            in_=qpt[blk][lo - blk * 128:hi - blk * 128, :],
                        )
                        dst += hi - lo
                logits_ps = lps.tile([128, 512], F32, tag="lg")
                for h in range(2):
                    z_ps = zps.tile([128, 1024], F32, tag="z")
                    for j2 in range(2):
                        kc = 2 * h + j2
                        nc.tensor.matmul(
                            z_ps[:, j2 * 512:j2 * 512 + R2],
                            rpw[0:KTOT, par * DM + kc * 128:
                                par * DM + (kc + 1) * 128],
                            xaT[0:KTOT, off:off + R2],
                            start=True, stop=True,
                        )
                    zt = zsb.tile([128, 2 * R2], F16, tag="zt")
                    nc.scalar.activation(
                        out=zt.rearrange("p (t c) -> p t c", t=2),
                        in_=z_ps.rearrange("p (t c) -> p t c", t=2)[:, :, 0:R2],
                        func=AF.Tanh,
                    )
                    for j2 in range(2):
                        kc = 2 * h + j2
                        nc.tensor.matmul(
                            logits_ps[:, 0:R2],
                            uarep[:, kc * 128:(kc + 1) * 128],
                            zt[:, j2 * R2:(j2 + 1) * R2],
                            start=(kc == 0), stop=(kc == KC - 1),
                        )
                e_sb = esb.tile([65, R2], F16, tag="e")
                nc.scalar.activation(
                    out=e_sb, in_=logits_ps[0:65, 0:R2], func=AF.Exp,
                    bias=eshift[0:65, :],
                )
                prod = esb.tile([65, R2], F16, tag="prod")
                nc.vector.tensor_tensor(
                    out=prod, in0=xaT[0:65, off:off + R2], in1=e_sb,
                    op=ALU.mult,
                )
                with nc.allow_low_precision(reason="fp16 prod"):
                    nc.vector.reduce_sum(
                        out=xaesum[0:65, b0:b0 + TB],
                        in_=prod.rearrange("p (g n) -> p g n", n=w),
                        axis=AX.X,
                    )

        # ---------------- final: normalize + output ----------------
        with ExitStack() as ctx:
            fps = ctx.enter_context(tc.tile_pool(name="fps", bufs=2, space="PSUM"))
            f2sb = ctx.enter_context(tc.tile_pool(name="f2sb", bufs=2))
            for blk in range(2):
                zc = f2sb.tile([128, 1], F32, tag="zc")
                nc.sync.dma_start(
                    out=zc, in_=xaesum[64:65, blk * 128:(blk + 1) * 128]
                )
                zc2 = f2sb.tile([128, 1], F32, tag="zc2")
                nc.vector.tensor_scalar(
                    out=zc2, in0=zc, scalar1=1e-30, scalar2=None, op0=ALU.add
                )
                nc.vector.reciprocal(out=recipz[:, blk:blk + 1], in_=zc2)
            for blk in range(2):
                out_ps = fps.tile([128, DM], F32, tag="op")
                nc.tensor.matmul(
                    out_ps, xaesum[0:65, blk * 128:(blk + 1) * 128], wb,
                    start=True, stop=True,
                )
                out_sb = f2sb.tile([128, DM], F32, tag="ob")
                nc.vector.tensor_scalar(
                    out=out_sb, in0=out_ps, scalar1=recipz[:, blk:blk + 1],
                    scalar2=None, op0=ALU.mult,
                )
                nc.sync.dma_start(
                    out=out.ap()[blk * 128:(blk + 1) * 128, :], in_=out_sb
                )

    nc.compile()
    return nc


def pack_inputs(x_others, x_mask):
    """Host-side layout prep: valid-first compaction + global sorted tiling."""
    mask_b = x_mask != 0
    c = mask_b.sum(1).astype(np.int64)
    order, plan, V = make_plan(c)
    # valid columns first within each batch (output is permutation-invariant)
    idx = np.argsort(~mask_b, axis=1, kind="stable")
    xm = (x_others.reshape(B, N, DOBJ).astype(np.float32)
          * x_mask[:, :, None].astype(np.float32)).astype(np.float16)
    xm_s = np.take_along_axis(xm, idx[:, :, None], axis=1)
    mk_s = np.take_along_axis(x_mask.astype(np.float16), idx, axis=1)

    indp = np.zeros((NIND, V), np.float16)
    for (TB, w, b0, off) in plan:
        for j in range(TB):
            indp[j, off + j * w:off + (j + 1) * w] = 1.0

    cores = []
    for k in range(NCORES):
        xmp = np.zeros((V, DOBJ), np.float16)
        mkp = np.zeros((1, V), np.float16)
        bl = np.empty(BSH, np.int64)
        p = 0
        for (TB, w, b0, off) in plan:
            for j in range(TB):
                g = order[p + k * TB + j]
                s = off + j * w
                xmp[s:s + w] = xm_s[g, :w]
                mkp[0, s:s + w] = mk_s[g, :w]
                bl[b0 + j] = g
            p += 8 * TB
        cores.append((np.ascontiguousarray(xmp.T), mkp, bl))
    return plan, V, indp, cores


def _ensure_ntff_hook():
    """Provide antenv.axon_hooks if the image lacks it (NTFF profiling via
    ctypes into libaxon_pjrt.so), and stub out the artifact upload."""
    import types
    import ctypes
    import contextlib

    try:
        from antenv.axon_hooks import get_axon_ntff_profile_hook  # noqa: F401
    except ImportError:
        so_path = "/opt/axon/libaxon_pjrt.so"
        hook = None
        if os.path.exists(so_path):
            lib = ctypes.CDLL(so_path)
            if hasattr(lib, "axon_start_nrt_profile"):
                lib.axon_start_nrt_profile.argtypes = [
                    ctypes.POINTER(ctypes.c_int64), ctypes.c_size_t,
                ]
                lib.axon_start_nrt_profile.restype = ctypes.c_int64
                lib.axon_stop_nrt_profile.argtypes = [ctypes.c_char_p]
                lib.axon_stop_nrt_profile.restype = ctypes.c_int64

                @contextlib.contextmanager
                def _hook(output_dir, device_ids):
                    import jax
                    jax.devices()
                    if device_ids:
                        ids = (ctypes.c_int64 * len(device_ids))(*device_ids)
                        rc = lib.axon_start_nrt_profile(ids, len(device_ids))
                    else:
                        rc = lib.axon_start_nrt_profile(None, 0)
                    if rc != 0:
                        raise RuntimeError(f"axon_start_nrt_profile rc={rc}")
                    try:
                        yield
                    finally:
                        n = lib.axon_stop_nrt_profile(str(output_dir).encode())
                        print(f"ntff profile: {n} file(s) -> {output_dir}",
                              file=sys.stderr)

                hook = _hook

        import antenv
        mod = types.ModuleType("antenv.axon_hooks")
        mod.get_axon_ntff_profile_hook = lambda: hook
        mod.set_axon_ntff_profile_hook = lambda h: None
        sys.modules["antenv.axon_hooks"] = mod
        antenv.axon_hooks = mod

    import concourse.bass_utils as bu
    bu.upload_artifacts = lambda tmpdir: f"file://{tmpdir}"


def kernel(x_others, x_mask, conv_w, conv_b, Uq, Ur, Ua):
    x_others = np.asarray(x_others)
    x_mask = np.asarray(x_mask)
    conv_w = np.ascontiguousarray(np.asarray(conv_w, dtype=np.float32))
    conv_b = np.asarray(conv_b, dtype=np.float32).reshape(1, DM)
    Uq = np.ascontiguousarray(np.asarray(Uq, dtype=np.float32))
    Ur = np.ascontiguousarray(np.asarray(Ur, dtype=np.float32))
    Ua = np.asarray(Ua, dtype=np.float32).reshape(1, DM)
    ident = np.eye(128, dtype=np.float32)

    plan, V, indp, cores = pack_inputs(x_others, x_mask)
    nc = build_nc(plan, V)

    in_maps = []
    for k in range(NCORES):
        xmt_k, mkp_k, _ = cores[k]
        in_maps.append({
            "xmt": xmt_k,
            "mask": mkp_k,
            "ind8": indp,
            "conv_w": conv_w,
            "conv_b": conv_b,
            "Uq": Uq,
            "Ur": Ur,
            "ua": Ua,
            "ident": ident,
        })

    from concourse.bass_utils import run_bass_kernel_spmd

    trace = os.environ.get("KERNEL_TRACE", "0") == "1"
    if trace:
        _ensure_ntff_hook()
    tmpdir = None
    if trace:
        import tempfile
        os.makedirs("/root/problem/traces", exist_ok=True)
        tmpdir = tempfile.mkdtemp(dir="/root/problem/traces")
        print(f"trace dir: {tmpdir}", file=sys.stderr)
    res = run_bass_kernel_spmd(
        nc, in_maps, core_ids=list(range(NCORES)), trace=trace, tmpdir=tmpdir
    )
    if trace and res.exec_time_ns is not None:
        print(f"HW exec time: {res.exec_time_ns} ns", file=sys.stderr)
        kernel.last_exec_time_ns = res.exec_time_ns
        kernel.last_trace = res.instructions_and_trace
    out = np.empty((B, DM), dtype=np.float32)
    for k, r in enumerate(res.results):
        out[cores[k][2]] = r["out"]
    return out


if __name__ == "__main__":
    rng = np.random.default_rng(0)
    x = rng.standard_normal((B, N * DOBJ), dtype=np.float32)
    mask = rng.integers(0, 2, (B, N)).astype(np.float32)
    w = rng.standard_normal((DOBJ, DM), dtype=np.float32) / 8.0
    cbv = np.zeros((DM,), dtype=np.float32)
    uq = rng.standard_normal((DM, DM), dtype=np.float32) / 22.6
    urm = rng.standard_normal((DM, DM), dtype=np.float32) / 22.6
    uav = rng.standard_normal((DM,), dtype=np.float32) * 0.1
    out = kernel(x, mask, w, cbv, uq, urm, uav)
    print(out.shape, out.dtype)
